# revision 2
# baseline (speedup 1.0000x reference)
"""Trainium2 Bass kernel for nn_Generator2 (2-layer GRU + fc, autoregressive).

Sharding: data-parallel over batch. 8 cores x 16 batch rows each; GRU/fc
weights replicated, SBUF-resident. Time recurrence sequential per core,
no cross-core communication.

Per step (per core, batch M=16), activations stationary on the PE, weights
streamed as the moving operand:
  r/z gate columns (2/3 of gate width): fp8 e4m3 + DoubleRow perf mode
  (2 K-rows per cycle) — weights pre-scaled by WS=2048, descale folded
  into the sigmoid's scale argument. Error-neutral vs bf16 (sigmoid
  attenuates the quantization noise).
  n gate columns: bf16 (precision-critical path into tanh).
Gate math in fp32 (PSUM + DVE/ACT); h carried in fp32; h^T kept as bf16
(for n-gate stationary) and fp8 (DoubleRow pairs for r/z stationary).
"""
import sys, types

import numpy as np

# ---------------------------------------------------------------------------
# environment shims (axon agent image)
# ---------------------------------------------------------------------------
try:
    import concourse.bass as bass  # noqa: F401
except ImportError:  # grading env may not have it on PYTHONPATH
    for p in ("/root/.axon_site", "/root/.axon_site/_ro/trn_rl_repo",
              "/root/.axon_site/_ro/pypackages", "/opt/trn_rl_repo"):
        if p not in sys.path:
            sys.path.append(p)
    import concourse.bass as bass

import antenv
if not hasattr(antenv, "axon_hooks"):
    _m = types.ModuleType("antenv.axon_hooks")
    _m._hook = None
    def _set(h): _m._hook = h
    def _get(): return _m._hook
    _m.set_axon_ntff_profile_hook = _set
    _m.get_axon_ntff_profile_hook = _get
    sys.modules["antenv.axon_hooks"] = _m
    antenv.axon_hooks = _m
    try:
        from trn_agent_boot.trn_boot import _ntff_profile_via_ctypes
        _m.set_axon_ntff_profile_hook(
            _ntff_profile_via_ctypes("/opt/axon/libaxon_pjrt.so"))
    except Exception:
        pass

import concourse.mybir as mybir
import concourse.tile as tile
from concourse.bacc import Bacc
from concourse.tile import ScopedClock
from concourse.bass_utils import run_bass_kernel_spmd

import ml_dtypes
BF16 = ml_dtypes.bfloat16
F8E4 = ml_dtypes.float8_e4m3

# walrus here accepts at most one sync-wait per instruction; split the Tile
# tail drain and post-process everything else.
_MAX_WAITS = 1

def _patched_drain_and_barrier(self, tick_clock, wait_clock):
    drain_inst = self.nc.sync.drain()
    wait_clock.add_sem_waits(drain_inst.ins,
                             ScopedClock({None: tick_clock.global_clock}))
    si = drain_inst.ins.sync_info
    if si is not None and si.on_wait and len(si.on_wait) > _MAX_WAITS:
        waits = list(si.on_wait)
        drain_inst.ins.sync_info = mybir.SyncInfo(
            on_wait=[waits[0]], on_update=list(si.on_update))
        for w in waits[1:]:
            d2 = self.nc.sync.drain()
            d2.ins.sync_info = mybir.SyncInfo(on_wait=[w], on_update=[])
    self.nc.all_engine_barrier()
    assert self.sems is not None
    popped = self.nc._tile_sem_poison_stack.pop()
    assert popped is self._sem_poison
    self.nc.clear_and_free_semaphores(list(self.sems.allocated().values()))
    self.nc.all_engine_barrier()

tile.TileContext._drain_and_barrier = _patched_drain_and_barrier

_split_ctr = [0]

def _split_excess_waits(nc, limit=_MAX_WAITS):
    for f in nc.m.functions:
        for bb in f.blocks:
            il = bb.instructions
            i = 0
            while i < len(il):
                ins = il[i]
                si = ins.sync_info
                if si is not None and si.on_wait and len(si.on_wait) > limit:
                    waits = list(si.on_wait)
                    extra, keep = waits[:-limit], waits[-limit:]
                    ins.sync_info = mybir.SyncInfo(
                        on_wait=keep, on_update=list(si.on_update))
                    for j, w in enumerate(extra):
                        _split_ctr[0] += 1
                        nop = mybir.InstNoOp(
                            name=f"I-wsplit-{_split_ctr[0]}",
                            engine=ins.engine,
                            bass_nofuse=True,
                            sync_info=mybir.SyncInfo(on_wait=[w], on_update=[]),
                        )
                        il.insert(i + j, nop)
                    i += len(extra)
                i += 1

# ---------------------------------------------------------------------------
# problem constants
# ---------------------------------------------------------------------------
B, T, C, H, O = 128, 256, 8, 1024, 2
IN = C + O                  # 10
G = 3 * H                   # 3072 gate width
RZ = 2 * H                  # 2048 r/z gate width
NCORES = 8
MB = B // NCORES            # 16 batch rows per core
UNROLL = 8                  # steps per For_i iteration
CH = 512
KT = H // 128               # 8 K tiles (bf16 n-path)
KP = H // 256               # 4 K pair-passes (fp8 DoubleRow rz-path)
WS = 2048.0                 # fp8 weight pre-scale (descaled in sigmoid)

F32 = mybir.dt.float32
BF = mybir.dt.bfloat16
FP8 = mybir.dt.float8e4
DRM = mybir.MatmulPerfMode.DoubleRow

AF = mybir.ActivationFunctionType
OP = mybir.AluOpType


def build(nT: int, unrolled: bool):
    nc = Bacc()
    P = lambda n, s, d: nc.declare_dram_parameter(n, s, d, isOutput=False)
    wih0 = P("wih0", [IN + 1, G], BF)          # moving for L0 gi (bias row; rz cols x WS)
    whh0rz = P("whh0rz", [KP, 128, 2, RZ], FP8)
    wih1rz = P("wih1rz", [KP, 128, 2, RZ], FP8)
    whh1rz = P("whh1rz", [KP, 128, 2, RZ], FP8)
    whh0n = P("whh0n", [KT, 128, H], BF)
    wih1n = P("wih1n", [KT, 128, H], BF)
    whh1n = P("whh1n", [KT, 128, H], BF)
    fcw = P("fcw", [KT, 128, O], BF)
    biasA1 = P("biasA1", [1, G], BF)           # rz cols x WS
    bhhn0 = P("bhhn0", [1, H], BF)
    bhhn1 = P("bhhn1", [1, H], BF)
    ones1 = P("ones1", [1, MB], BF)
    fcb = P("fcb", [MB, O], F32)
    nU = max(nT // UNROLL, 1)
    xseq = P("xseq", [UNROLL, C, nU * MB], BF)  # [j][:, i] = x for step U*i+j+1
    inp0 = P("inp0", [IN + 1, MB], BF)
    ident = P("ident", [MB, MB], F32)
    y_out = nc.declare_dram_parameter(
        "y_out", [UNROLL, MB, nU * O], F32, isOutput=True)

    with tile.TileContext(nc) as tc:
        with (
            tc.tile_pool(name="const", bufs=1) as cpool,
            tc.tile_pool(name="state", bufs=1) as spool,
            tc.tile_pool(name="work", bufs=3) as wpool,
            tc.tile_pool(name="psum", bufs=8, space="PSUM") as psum,
        ):
            # --- constants / weights in SBUF -----------------------------
            wih0_sb = cpool.tile([IN + 1, G], BF, tag="wih0")
            whh0rz_sb = cpool.tile([128, KP, 2, RZ], FP8, tag="whh0rz")
            wih1rz_sb = cpool.tile([128, KP, 2, RZ], FP8, tag="wih1rz")
            whh1rz_sb = cpool.tile([128, KP, 2, RZ], FP8, tag="whh1rz")
            whh0n_sb = cpool.tile([128, KT, H], BF, tag="whh0n")
            wih1n_sb = cpool.tile([128, KT, H], BF, tag="wih1n")
            whh1n_sb = cpool.tile([128, KT, H], BF, tag="whh1n")
            fcw_sb = cpool.tile([128, KT * O], BF, tag="fcw")
            biasA1_sb = cpool.tile([1, G], BF, tag="biasA1")
            bhhn0_sb = cpool.tile([1, H], BF, tag="bhhn0")
            bhhn1_sb = cpool.tile([1, H], BF, tag="bhhn1")
            ones1_sb = cpool.tile([1, MB], BF, tag="ones1")
            fcb_sb = cpool.tile([MB, O], F32, tag="fcb")
            xseq_sb = [cpool.tile([C, nU * MB], BF, tag=f"xseq{j}",
                                  name=f"xseq{j}") for j in range(UNROLL)]
            ident_sb = cpool.tile([MB, MB], F32, tag="ident")

            nc.sync.dma_start(wih0_sb[:], wih0[:])
            for p in range(KP):
                nc.sync.dma_start(whh0rz_sb[:, p, :, :], whh0rz[p])
                nc.sync.dma_start(wih1rz_sb[:, p, :, :], wih1rz[p])
                nc.sync.dma_start(whh1rz_sb[:, p, :, :], whh1rz[p])
            for k in range(KT):
                nc.sync.dma_start(whh0n_sb[:, k, :], whh0n[k])
                nc.sync.dma_start(wih1n_sb[:, k, :], wih1n[k])
                nc.sync.dma_start(whh1n_sb[:, k, :], whh1n[k])
                nc.sync.dma_start(fcw_sb[:, k * O:(k + 1) * O], fcw[k])
            nc.sync.dma_start(biasA1_sb[:], biasA1[:])
            nc.sync.dma_start(ones1_sb[:], ones1[:])
            nc.sync.dma_start(bhhn0_sb[:], bhhn0[:])
            nc.sync.dma_start(bhhn1_sb[:], bhhn1[:])
            nc.sync.dma_start(fcb_sb[:], fcb[:])
            for j in range(UNROLL):
                nc.sync.dma_start(xseq_sb[j][:], xseq[j])
            nc.sync.dma_start(ident_sb[:], ident[:])

            # --- state ---------------------------------------------------
            h_sb = [spool.tile([MB, H], F32, tag=f"h{l}", name=f"h{l}")
                    for l in range(2)]
            hT_sb = [spool.tile([128, KT, MB], BF, tag=f"hT{l}",
                                name=f"hT{l}") for l in range(2)]
            hT8_sb = [spool.tile([128, KT, MB], FP8, tag=f"hT8{l}",
                                 name=f"hT8{l}") for l in range(2)]
            y_acc = [spool.tile([MB, nU * O], F32, tag=f"y_acc{j}",
                                name=f"y_acc{j}") for j in range(UNROLL)]
            inp_cur = spool.tile([IN + 1, MB], BF, tag="inp_cur")
            y_cur = spool.tile([MB, O], F32, tag="y_cur")

            # t=0 input + ones row from host
            nc.sync.dma_start(inp_cur[:], inp0[:])

            for l in range(2):
                nc.vector.memset(h_sb[l][:], 0.0)
                nc.vector.memset(hT_sb[l][:], 0.0)
                nc.vector.memset(hT8_sb[l][:], 0.0)

            # ---- per-layer phase emitters (explicit PE-order control) ----
            def p1_alloc():
                rt = psum.tile([MB, 2 * CH], F32, tag="pb", bufs=4, name="rt")
                zt = psum.tile([MB, 2 * CH], F32, tag="pb", bufs=4, name="zt")
                return rt, zt

            def p1_B(lidx, rt, zt):
                """h-dependent part of r/z gates: fp8 DoubleRow streams."""
                halves = [rt[:, 0:CH], rt[:, CH:2 * CH],
                          zt[:, 0:CH], zt[:, CH:2 * CH]]
                wrz = whh0rz_sb if lidx == 0 else whh1rz_sb
                for p in range(KP):
                    for c in range(4):
                        nc.tensor.matmul(
                            halves[c], hT8_sb[lidx][:, 2 * p:2 * p + 2, :],
                            wrz[:, p, :, c * CH:(c + 1) * CH],
                            start=(p == 0), stop=False, perf_mode=DRM)
                if lidx == 1:
                    for c in range(4):
                        nc.tensor.matmul(
                            halves[c], ones1_sb[:],
                            biasA1_sb[:, c * CH:(c + 1) * CH],
                            start=False, stop=False)

            def p1_A(lidx, rt, zt):
                """input-dependent part of r/z gates (stop instructions)."""
                halves = [rt[:, 0:CH], rt[:, CH:2 * CH],
                          zt[:, 0:CH], zt[:, CH:2 * CH]]
                if lidx == 0:
                    for c in range(4):
                        nc.tensor.matmul(
                            halves[c], inp_cur[:],
                            wih0_sb[:, c * CH:(c + 1) * CH],
                            start=False, stop=True)
                else:
                    for p in range(KP):
                        for c in range(4):
                            nc.tensor.matmul(
                                halves[c], hT8_sb[0][:, 2 * p:2 * p + 2, :],
                                wih1rz_sb[:, p, :, c * CH:(c + 1) * CH],
                                start=False, stop=(p == KP - 1),
                                perf_mode=DRM)

            def sigmoids(lidx, rt, zt):
                r_sb = wpool.tile([MB, H], BF, tag=f"r{lidx}", bufs=1,
                                  name=f"r{lidx}")
                z_sb = wpool.tile([MB, H], BF, tag=f"z{lidx}", bufs=1,
                                  name=f"z{lidx}")
                nc.scalar.activation(r_sb[:], rt[:], AF.Sigmoid, scale=1.0 / WS)
                nc.scalar.activation(z_sb[:], zt[:], AF.Sigmoid, scale=1.0 / WS)
                return r_sb, z_sb

            def p2_B(lidx):
                An = psum.tile([MB, 2 * CH], F32, tag="pb", bufs=4, name="An")
                Bn = psum.tile([MB, 2 * CH], F32, tag="pb", bufs=4, name="Bn")
                wn = whh0n_sb if lidx == 0 else whh1n_sb
                bhhn_sb = bhhn0_sb if lidx == 0 else bhhn1_sb
                for k in range(KT):
                    for c in range(2):
                        nc.tensor.matmul(
                            Bn[:, c * CH:(c + 1) * CH],
                            hT_sb[lidx][:, k, :],
                            wn[:, k, c * CH:(c + 1) * CH],
                            start=(k == 0), stop=False)
                for c in range(2):
                    nc.tensor.matmul(
                        Bn[:, c * CH:(c + 1) * CH], ones1_sb[:],
                        bhhn_sb[:, c * CH:(c + 1) * CH],
                        start=False, stop=True)
                return An, Bn

            def p2_A(lidx, An):
                if lidx == 0:
                    for c in range(2):
                        nc.tensor.matmul(
                            An[:, c * CH:(c + 1) * CH], inp_cur[:],
                            wih0_sb[:, RZ + c * CH: RZ + (c + 1) * CH],
                            start=True, stop=True)
                else:
                    for k in range(KT):
                        for c in range(2):
                            nc.tensor.matmul(
                                An[:, c * CH:(c + 1) * CH],
                                hT_sb[0][:, k, :],
                                wih1n_sb[:, k, c * CH:(c + 1) * CH],
                                start=(k == 0), stop=False)
                    for c in range(2):
                        nc.tensor.matmul(
                            An[:, c * CH:(c + 1) * CH], ones1_sb[:],
                            biasA1_sb[:, RZ + c * CH: RZ + (c + 1) * CH],
                            start=False, stop=True)

            def n_chain(lidx, r_sb, z_sb, An, Bn):
                h = h_sb[lidx]
                v = wpool.tile([MB, H], F32, tag="gtmp", bufs=1)
                nc.vector.tensor_mul(v[:], r_sb[:], Bn[:])
                nc.vector.tensor_add(v[:], v[:], An[:])
                nn = wpool.tile([MB, H], F32, tag="nn", bufs=1)
                nc.scalar.activation(nn[:], v[:], AF.Tanh)
                d = wpool.tile([MB, H], F32, tag="dtmp", bufs=1)
                nc.vector.tensor_sub(d[:], h[:], nn[:])
                nc.vector.tensor_mul(d[:], z_sb[:], d[:])
                nc.vector.tensor_add(h[:], d[:], nn[:])

            def transposes(lidx):
                h = h_sb[lidx]
                hb = wpool.tile([MB, H], BF, tag="hb", bufs=2)
                nc.vector.tensor_copy(hb[:], h[:])
                for k in range(KT):
                    nc.sync.dma_start_transpose(
                        hT_sb[lidx][:, k, :], hb[:, k * 128:(k + 1) * 128])
                # fp8 copy of h^T for the DoubleRow r/z stationary
                nc.gpsimd.tensor_copy(hT8_sb[lidx][:], hT_sb[lidx][:])

            def y_tail(j, t):
                yp = psum.tile([MB, O], F32, tag="pb", bufs=4, name="ypb")
                for k in range(KT):
                    nc.tensor.matmul(
                        yp[:], hT_sb[1][:, k, :],
                        fcw_sb[:, k * O:(k + 1) * O],
                        start=(k == 0), stop=(k == KT - 1))
                ytmp = wpool.tile([MB, O], F32, tag="ytmp", bufs=2)
                nc.vector.tensor_add(ytmp[:], yp[:], fcb_sb[:])
                nc.scalar.activation(y_cur[:], ytmp[:], AF.Tanh)
                nc.vector.tensor_copy(y_acc[j][:, bass.ts(t, O)], y_cur[:])
                typ = psum.tile([O, MB], F32, tag="pb", bufs=4, name="typb")
                nc.tensor.transpose(typ[:], y_cur[:], ident_sb[:])
                nc.vector.tensor_copy(inp_cur[0:O, :], typ[:])
                nc.sync.dma_start(
                    inp_cur[O:IN, :], xseq_sb[j][:, bass.ts(t, MB)])

            def emit_body(t_of):
                """Emit UNROLL steps with cross-step interleaved PE order."""
                nxt = [None]  # rt/zt preallocated+B-filled for next step L0
                def l0_p1B():
                    rt, zt = p1_alloc()
                    p1_B(0, rt, zt)
                    return rt, zt
                nxt[0] = l0_p1B()
                for j in range(UNROLL):
                    rt0, zt0 = nxt[0]
                    p1_A(0, rt0, zt0)
                    r0, z0 = sigmoids(0, rt0, zt0)
                    An0, Bn0 = p2_B(0)
                    p2_A(0, An0)
                    rt1, zt1 = p1_alloc()
                    p1_B(1, rt1, zt1)             # fills L0 n-chain latency
                    n_chain(0, r0, z0, An0, Bn0)
                    transposes(0)
                    p1_A(1, rt1, zt1)
                    r1, z1 = sigmoids(1, rt1, zt1)
                    An1, Bn1 = p2_B(1)
                    p2_A(1, An1)
                    if j + 1 < UNROLL:
                        nxt[0] = l0_p1B()          # fills L1 n-chain latency
                    n_chain(1, r1, z1, An1, Bn1)
                    transposes(1)
                    y_tail(j, t_of(j))

            if unrolled:
                assert nT <= UNROLL, "unrolled mode supports nT <= UNROLL"
                emit_body(lambda j: 0)
            else:
                assert nT % UNROLL == 0
                with tc.For_i(0, nT // UNROLL, 1,
                              hint_engines=(mybir.EngineType.PE,
                                            mybir.EngineType.DVE,
                                            mybir.EngineType.Activation)) as iv:
                    emit_body(lambda j: iv)

            for j in range(UNROLL):
                nc.sync.dma_start(y_out[j], y_acc[j][:])

    return nc


# ---------------------------------------------------------------------------
# host-side preparation
# ---------------------------------------------------------------------------

def _pair_fp8(wT_rz):
    """[H, RZ] f32 -> [KP, 128, 2, RZ] fp8 DoubleRow pair layout.

    pass p partition j pairs K rows (256p + j, 256p + 128 + j)."""
    out = np.zeros((KP, 128, 2, RZ), np.float32)
    for p in range(KP):
        blk = wT_rz[256 * p:256 * (p + 1)]           # [256, RZ]
        out[p, :, 0, :] = blk[0:128]
        out[p, :, 1, :] = blk[128:256]
    return np.asarray(out * WS, F8E4)


def _prep_in_maps(x, init_noise, w_ih0, w_hh0, b_ih0, b_hh0,
                  w_ih1, w_hh1, b_ih1, b_hh1, fc_w, fc_b, nT):
    f32 = np.float32
    x = np.asarray(x, f32)
    init_noise = np.asarray(init_noise, f32)
    w_ih0 = np.asarray(w_ih0, f32); w_hh0 = np.asarray(w_hh0, f32)
    b_ih0 = np.asarray(b_ih0, f32); b_hh0 = np.asarray(b_hh0, f32)
    w_ih1 = np.asarray(w_ih1, f32); w_hh1 = np.asarray(w_hh1, f32)
    b_ih1 = np.asarray(b_ih1, f32); b_hh1 = np.asarray(b_hh1, f32)
    fc_w = np.asarray(fc_w, f32); fc_b = np.asarray(fc_b, f32)

    # L0 moving operand with bias row: rz bias = b_ih0+b_hh0, n bias = b_ih0.
    # rz columns (and their bias entries) pre-scaled by WS to match the fp8
    # DoubleRow gh0 products accumulated into the same PSUM.
    bias0 = np.concatenate([(b_ih0 + b_hh0)[:RZ], b_ih0[RZ:]])
    wih0_m = np.concatenate([w_ih0.T, bias0[None, :]], 0)        # [11, G]
    wih0_m = wih0_m.copy()
    wih0_m[:, :RZ] *= WS

    whh0rz = _pair_fp8(w_hh0.T[:, :RZ])
    wih1rz = _pair_fp8(w_ih1.T[:, :RZ])
    whh1rz = _pair_fp8(w_hh1.T[:, :RZ])
    whh0n = w_hh0.T[:, RZ:].reshape(KT, 128, H)
    wih1n = w_ih1.T[:, RZ:].reshape(KT, 128, H)
    whh1n = w_hh1.T[:, RZ:].reshape(KT, 128, H)
    fcw_m = fc_w.T.reshape(KT, 128, O)
    biasA1 = np.concatenate(
        [(b_ih1 + b_hh1)[:RZ] * WS, b_ih1[RZ:]])[None, :]
    bhhn0 = b_hh0[RZ:][None, :]
    bhhn1 = b_hh1[RZ:][None, :]
    ones1 = np.ones((1, MB), f32)
    fcb = np.tile(fc_b[None, :], (MB, 1))
    ident = np.eye(MB, dtype=f32)

    noise = init_noise.copy()
    noise[:, -4:] = x[:, 0, -4:]

    in_maps = []
    for c in range(NCORES):
        sl = slice(c * MB, (c + 1) * MB)
        xs = x[sl]            # [MB, T, C]
        # buffer j slot i holds the x-part consumed by step U*i+j+1
        nU = max(nT // UNROLL, 1)
        xseq_h = np.zeros((UNROLL, C, nU * MB), f32)
        for t in range(nT):
            i, j = t // UNROLL, t % UNROLL
            xseq_h[j][:, i * MB:(i + 1) * MB] = xs[:, t, :].T
        inp0v = np.zeros((IN + 1, MB), f32)
        inp0v[IN, :] = 1.0
        inp0v[0:O, :] = 1.0
        inp0v[O:IN, :] = noise[sl].T
        in_maps.append({
            "wih0": wih0_m.astype(BF16),
            "whh0rz": whh0rz, "wih1rz": wih1rz, "whh1rz": whh1rz,
            "whh0n": whh0n.astype(BF16), "wih1n": wih1n.astype(BF16),
            "whh1n": whh1n.astype(BF16),
            "fcw": fcw_m.astype(BF16),
            "biasA1": biasA1.astype(BF16), "bhhn0": bhhn0.astype(BF16),
            "bhhn1": bhhn1.astype(BF16), "ones1": ones1.astype(BF16),
            "fcb": fcb,
            "xseq": xseq_h.astype(BF16), "inp0": inp0v.astype(BF16),
            "ident": ident,
        })
    return in_maps


_BUILD_CACHE = {}

def _get_nc(nT, unrolled):
    key = (nT, unrolled)
    if key not in _BUILD_CACHE:
        nc = build(nT, unrolled)
        nc.finalize()
        _split_excess_waits(nc)
        _BUILD_CACHE[key] = nc
    return _BUILD_CACHE[key]


def run(nT=T, unrolled=False, trace=False, **inputs):
    inputs.pop("xlens", None)
    in_maps = _prep_in_maps(nT=nT, **inputs)
    nc = _get_nc(nT, unrolled)
    res = run_bass_kernel_spmd(nc, in_maps, list(range(NCORES)), trace=trace)
    out = np.zeros((B, nT, O), np.float32)
    nU = max(nT // UNROLL, 1)
    for c in range(NCORES):
        yo = res.results[c]["y_out"].reshape(UNROLL, MB, nU, O)
        for j in range(UNROLL):
            out[c * MB:(c + 1) * MB, j::UNROLL] = yo[j].transpose(0, 1, 2)
    return out, res


def kernel(**inputs):
    out, _ = run(nT=T, unrolled=False, **inputs)
    return out


# revision 16
# speedup vs baseline: 1.1693x; 1.1693x over previous
"""Trainium2 Bass kernel for nn_Generator2 (2-layer GRU + fc, autoregressive).

Sharding: data-parallel over batch. 8 cores x 16 batch rows each; GRU/fc
weights replicated, SBUF-resident. Time recurrence sequential per core,
no cross-core communication.

Per step (per core, batch M=16), activations stationary on the PE, weights
streamed as the moving operand:
  r/z gate columns (2/3 of gate width): fp8 e4m3 + DoubleRow perf mode
  (2 K-rows per cycle) — weights pre-scaled by WS=2048, descale folded
  into the sigmoid's scale argument. Error-neutral vs bf16 (sigmoid
  attenuates the quantization noise).
  n gate columns: bf16 (precision-critical path into tanh).
Gate math in fp32 (PSUM + DVE/ACT); h carried in fp32; h^T kept as bf16
(for n-gate stationary) and fp8 (DoubleRow pairs for r/z stationary).
"""
import sys, types

import numpy as np

# ---------------------------------------------------------------------------
# environment shims (axon agent image)
# ---------------------------------------------------------------------------
try:
    import concourse.bass as bass  # noqa: F401
except ImportError:  # grading env may not have it on PYTHONPATH
    for p in ("/root/.axon_site", "/root/.axon_site/_ro/trn_rl_repo",
              "/root/.axon_site/_ro/pypackages", "/opt/trn_rl_repo"):
        if p not in sys.path:
            sys.path.append(p)
    import concourse.bass as bass

import antenv
if not hasattr(antenv, "axon_hooks"):
    _m = types.ModuleType("antenv.axon_hooks")
    _m._hook = None
    def _set(h): _m._hook = h
    def _get(): return _m._hook
    _m.set_axon_ntff_profile_hook = _set
    _m.get_axon_ntff_profile_hook = _get
    sys.modules["antenv.axon_hooks"] = _m
    antenv.axon_hooks = _m
    try:
        from trn_agent_boot.trn_boot import _ntff_profile_via_ctypes
        _m.set_axon_ntff_profile_hook(
            _ntff_profile_via_ctypes("/opt/axon/libaxon_pjrt.so"))
    except Exception:
        pass

import concourse.mybir as mybir
import concourse.tile as tile
from concourse.bacc import Bacc
from concourse.tile import ScopedClock
from concourse.bass_utils import run_bass_kernel_spmd

import ml_dtypes
BF16 = ml_dtypes.bfloat16
F8E4 = ml_dtypes.float8_e4m3

# walrus here accepts at most one sync-wait per instruction; split the Tile
# tail drain and post-process everything else.
_MAX_WAITS = 1

def _patched_drain_and_barrier(self, tick_clock, wait_clock):
    drain_inst = self.nc.sync.drain()
    wait_clock.add_sem_waits(drain_inst.ins,
                             ScopedClock({None: tick_clock.global_clock}))
    si = drain_inst.ins.sync_info
    if si is not None and si.on_wait and len(si.on_wait) > _MAX_WAITS:
        waits = list(si.on_wait)
        drain_inst.ins.sync_info = mybir.SyncInfo(
            on_wait=[waits[0]], on_update=list(si.on_update))
        for w in waits[1:]:
            d2 = self.nc.sync.drain()
            d2.ins.sync_info = mybir.SyncInfo(on_wait=[w], on_update=[])
    self.nc.all_engine_barrier()
    assert self.sems is not None
    popped = self.nc._tile_sem_poison_stack.pop()
    assert popped is self._sem_poison
    self.nc.clear_and_free_semaphores(list(self.sems.allocated().values()))
    self.nc.all_engine_barrier()

tile.TileContext._drain_and_barrier = _patched_drain_and_barrier

_split_ctr = [0]

def _split_excess_waits(nc, limit=_MAX_WAITS):
    for f in nc.m.functions:
        for bb in f.blocks:
            il = bb.instructions
            i = 0
            while i < len(il):
                ins = il[i]
                si = ins.sync_info
                if si is not None and si.on_wait and len(si.on_wait) > limit:
                    waits = list(si.on_wait)
                    extra, keep = waits[:-limit], waits[-limit:]
                    ins.sync_info = mybir.SyncInfo(
                        on_wait=keep, on_update=list(si.on_update))
                    for j, w in enumerate(extra):
                        _split_ctr[0] += 1
                        nop = mybir.InstNoOp(
                            name=f"I-wsplit-{_split_ctr[0]}",
                            engine=ins.engine,
                            bass_nofuse=True,
                            sync_info=mybir.SyncInfo(on_wait=[w], on_update=[]),
                        )
                        il.insert(i + j, nop)
                    i += len(extra)
                i += 1

# ---------------------------------------------------------------------------
# problem constants
# ---------------------------------------------------------------------------
B, T, C, H, O = 128, 256, 8, 1024, 2
IN = C + O                  # 10
G = 3 * H                   # 3072 gate width
RZ = 2 * H                  # 2048 r/z gate width
NCORES = 8
MB = B // NCORES            # 16 batch rows per core
UNROLL = 8                  # steps per For_i iteration
CH = 512
KT = H // 128               # 8 K tiles (bf16 n-path)
KP = H // 256               # 4 K pair-passes (fp8 DoubleRow rz-path)
WS = 2048.0                 # fp8 weight pre-scale (descaled in sigmoid)

F32 = mybir.dt.float32
BF = mybir.dt.bfloat16
FP8 = mybir.dt.float8e4
DRM = mybir.MatmulPerfMode.DoubleRow

AF = mybir.ActivationFunctionType
OP = mybir.AluOpType


def build(nT: int, unrolled: bool):
    nc = Bacc()
    P = lambda n, s, d: nc.declare_dram_parameter(n, s, d, isOutput=False)
    wih0 = P("wih0", [IN + 1, G], BF)          # moving for L0 gi (bias row; rz cols x WS)
    whh0rz = P("whh0rz", [KP, 128, 2, RZ], FP8)
    wih1rz = P("wih1rz", [KP, 128, 2, RZ], FP8)
    whh1rz = P("whh1rz", [KP, 128, 2, RZ], FP8)
    whh0n = P("whh0n", [KT, 128, H], BF)
    wih1n = P("wih1n", [KT, 128, H], BF)
    whh1n = P("whh1n", [KT, 128, H], BF)
    fcw = P("fcw", [KT, 128, O], BF)
    biasA1 = P("biasA1", [1, G], BF)           # rz cols x WS
    bhhn0 = P("bhhn0", [1, H], BF)
    bhhn1 = P("bhhn1", [1, H], BF)
    ones1 = P("ones1", [1, MB], BF)
    fcb = P("fcb", [MB, O], F32)
    nU = max(nT // UNROLL, 1)
    xseq = P("xseq", [UNROLL, C, nU * MB], BF)  # [j][:, i] = x for step U*i+j+1
    inp0 = P("inp0", [IN + 1, MB], BF)
    ident = P("ident", [MB, MB], F32)
    y_out = nc.declare_dram_parameter(
        "y_out", [UNROLL, MB, nU * O], F32, isOutput=True)

    with tile.TileContext(nc) as tc:
        with (
            tc.tile_pool(name="const", bufs=1) as cpool,
            tc.tile_pool(name="state", bufs=1) as spool,
            tc.tile_pool(name="work", bufs=3) as wpool,
            tc.tile_pool(name="psum", bufs=8, space="PSUM") as psum,
        ):
            # --- constants / weights in SBUF -----------------------------
            wih0_sb = cpool.tile([IN + 1, G], BF, tag="wih0")
            whh0rz_sb = cpool.tile([128, KP, 2, RZ], FP8, tag="whh0rz")
            wih1rz_sb = cpool.tile([128, KP, 2, RZ], FP8, tag="wih1rz")
            whh1rz_sb = cpool.tile([128, KP, 2, RZ], FP8, tag="whh1rz")
            whh0n_sb = cpool.tile([128, KT, H], BF, tag="whh0n")
            wih1n_sb = cpool.tile([128, KT, H], BF, tag="wih1n")
            whh1n_sb = cpool.tile([128, KT, H], BF, tag="whh1n")
            fcw_sb = cpool.tile([128, KT * O], BF, tag="fcw")
            biasA1_sb = cpool.tile([1, G], BF, tag="biasA1")
            bhhn0_sb = cpool.tile([1, H], BF, tag="bhhn0")
            bhhn1_sb = cpool.tile([1, H], BF, tag="bhhn1")
            ones1_sb = cpool.tile([1, MB], BF, tag="ones1")
            fcb_sb = cpool.tile([MB, O], F32, tag="fcb")
            xseq_sb = [cpool.tile([C, nU * MB], BF, tag=f"xseq{j}",
                                  name=f"xseq{j}") for j in range(UNROLL)]
            ident_sb = cpool.tile([MB, MB], F32, tag="ident")

            nc.sync.dma_start(wih0_sb[:], wih0[:])
            for p in range(KP):
                nc.sync.dma_start(whh0rz_sb[:, p, :, :], whh0rz[p])
                nc.sync.dma_start(wih1rz_sb[:, p, :, :], wih1rz[p])
                nc.sync.dma_start(whh1rz_sb[:, p, :, :], whh1rz[p])
            for k in range(KT):
                nc.sync.dma_start(whh0n_sb[:, k, :], whh0n[k])
                nc.sync.dma_start(wih1n_sb[:, k, :], wih1n[k])
                nc.sync.dma_start(whh1n_sb[:, k, :], whh1n[k])
                nc.sync.dma_start(fcw_sb[:, k * O:(k + 1) * O], fcw[k])
            nc.sync.dma_start(biasA1_sb[:], biasA1[:])
            nc.sync.dma_start(ones1_sb[:], ones1[:])
            nc.sync.dma_start(bhhn0_sb[:], bhhn0[:])
            nc.sync.dma_start(bhhn1_sb[:], bhhn1[:])
            nc.sync.dma_start(fcb_sb[:], fcb[:])
            for j in range(UNROLL):
                nc.sync.dma_start(xseq_sb[j][:], xseq[j])
            nc.sync.dma_start(ident_sb[:], ident[:])

            # --- state ---------------------------------------------------
            h_sb = [spool.tile([MB, H], F32, tag=f"h{l}", name=f"h{l}")
                    for l in range(2)]
            hT_sb = [spool.tile([128, KT, MB], BF, tag=f"hT{l}",
                                name=f"hT{l}") for l in range(2)]
            hT8_sb = [spool.tile([128, KT, MB], FP8, tag=f"hT8{l}",
                                 name=f"hT8{l}") for l in range(2)]
            y_acc = [spool.tile([MB, nU * O], F32, tag=f"y_acc{j}",
                                name=f"y_acc{j}") for j in range(UNROLL)]
            inp_cur = spool.tile([IN + 1, MB], BF, tag="inp_cur")
            y_cur = spool.tile([MB, O], F32, tag="y_cur")

            # t=0 input + ones row from host
            nc.sync.dma_start(inp_cur[:], inp0[:])

            for l in range(2):
                nc.vector.memset(h_sb[l][:], 0.0)
                nc.vector.memset(hT_sb[l][:], 0.0)
                nc.vector.memset(hT8_sb[l][:], 0.0)

            # ---- per-layer phase emitters (explicit PE-order control) ----
            def p1_alloc():
                rt = psum.tile([MB, 2 * CH], F32, tag="pb", bufs=4, name="rt")
                zt = psum.tile([MB, 2 * CH], F32, tag="pb", bufs=4, name="zt")
                return rt, zt

            def p1_B(lidx, rt, zt):
                """h-dependent part of r/z gates: fp8 DoubleRow streams."""
                halves = [rt[:, 0:CH], rt[:, CH:2 * CH],
                          zt[:, 0:CH], zt[:, CH:2 * CH]]
                wrz = whh0rz_sb if lidx == 0 else whh1rz_sb
                for p in range(KP):
                    for c in range(4):
                        nc.tensor.matmul(
                            halves[c], hT8_sb[lidx][:, 2 * p:2 * p + 2, :],
                            wrz[:, p, :, c * CH:(c + 1) * CH],
                            start=(p == 0), stop=False, perf_mode=DRM)
                if lidx == 1:
                    for c in range(4):
                        nc.tensor.matmul(
                            halves[c], ones1_sb[:],
                            biasA1_sb[:, c * CH:(c + 1) * CH],
                            start=False, stop=False)

            def p1_A(lidx, rt, zt):
                """input-dependent part of r/z gates (stop instructions)."""
                halves = [rt[:, 0:CH], rt[:, CH:2 * CH],
                          zt[:, 0:CH], zt[:, CH:2 * CH]]
                if lidx == 0:
                    for c in range(4):
                        nc.tensor.matmul(
                            halves[c], inp_cur[:],
                            wih0_sb[:, c * CH:(c + 1) * CH],
                            start=False, stop=True)
                else:
                    for p in range(KP):
                        for c in range(4):
                            nc.tensor.matmul(
                                halves[c], hT8_sb[0][:, 2 * p:2 * p + 2, :],
                                wih1rz_sb[:, p, :, c * CH:(c + 1) * CH],
                                start=False, stop=(p == KP - 1),
                                perf_mode=DRM)

            def sigmoids(lidx, rt, zt):
                r_sb = wpool.tile([MB, H], BF, tag=f"r{lidx}", bufs=1,
                                  name=f"r{lidx}")
                z_sb = wpool.tile([MB, H], BF, tag=f"z{lidx}", bufs=1,
                                  name=f"z{lidx}")
                nc.scalar.activation(r_sb[:], rt[:], AF.Sigmoid, scale=1.0 / WS)
                nc.scalar.activation(z_sb[:], zt[:], AF.Sigmoid, scale=1.0 / WS)
                return r_sb, z_sb

            def p2_B(lidx):
                An = psum.tile([MB, 2 * CH], F32, tag="pb", bufs=4, name="An")
                Bn = psum.tile([MB, 2 * CH], F32, tag="pb", bufs=4, name="Bn")
                wn = whh0n_sb if lidx == 0 else whh1n_sb
                bhhn_sb = bhhn0_sb if lidx == 0 else bhhn1_sb
                for k in range(KT):
                    for c in range(2):
                        nc.tensor.matmul(
                            Bn[:, c * CH:(c + 1) * CH],
                            hT_sb[lidx][:, k, :],
                            wn[:, k, c * CH:(c + 1) * CH],
                            start=(k == 0), stop=False)
                for c in range(2):
                    nc.tensor.matmul(
                        Bn[:, c * CH:(c + 1) * CH], ones1_sb[:],
                        bhhn_sb[:, c * CH:(c + 1) * CH],
                        start=False, stop=True)
                return An, Bn

            def p2_A(lidx, An):
                if lidx == 0:
                    for c in range(2):
                        nc.tensor.matmul(
                            An[:, c * CH:(c + 1) * CH], inp_cur[:],
                            wih0_sb[:, RZ + c * CH: RZ + (c + 1) * CH],
                            start=True, stop=True)
                else:
                    for k in range(KT):
                        for c in range(2):
                            nc.tensor.matmul(
                                An[:, c * CH:(c + 1) * CH],
                                hT_sb[0][:, k, :],
                                wih1n_sb[:, k, c * CH:(c + 1) * CH],
                                start=(k == 0), stop=False)
                    for c in range(2):
                        nc.tensor.matmul(
                            An[:, c * CH:(c + 1) * CH], ones1_sb[:],
                            biasA1_sb[:, RZ + c * CH: RZ + (c + 1) * CH],
                            start=False, stop=True)

            # n_chain: PSUM-reading ops on DVE (chunked so tanh starts
            # early); post-tanh SBUF-only tail split across DVE/GpSimd.
            NSPL = 640

            def n_chain(lidx, r_sb, z_sb, An, Bn):
                h = h_sb[lidx]
                v = wpool.tile([MB, H], F32, tag="gtmp", bufs=1)
                nn = wpool.tile([MB, H], F32, tag="nn", bufs=1)
                d = wpool.tile([MB, H], F32, tag="dtmp", bufs=1)
                sa, sb_ = slice(0, NSPL), slice(NSPL, H)
                for sl in (sa, sb_):
                    nc.vector.tensor_mul(v[:, sl], r_sb[:, sl], Bn[:, sl])
                    nc.vector.tensor_add(v[:, sl], v[:, sl], An[:, sl])
                    nc.scalar.activation(nn[:, sl], v[:, sl], AF.Tanh)
                for eng, sl in ((nc.vector, sa), (nc.gpsimd, sb_)):
                    eng.tensor_sub(d[:, sl], h[:, sl], nn[:, sl])
                    eng.tensor_mul(d[:, sl], z_sb[:, sl], d[:, sl])
                    eng.tensor_add(h[:, sl], d[:, sl], nn[:, sl])

            def transposes(lidx):
                h = h_sb[lidx]
                hb = wpool.tile([MB, H], BF, tag="hb", bufs=2)
                nc.vector.tensor_copy(hb[:], h[:])
                # spread the 8 transposes over both HWDGE-capable queues
                # (SP + Activation): ~1.2us serialized issue each, so
                # 2-way halves the critical-path latency.
                engs = (nc.sync, nc.scalar)
                for k in range(KT):
                    engs[k % 2].dma_start_transpose(
                        hT_sb[lidx][:, k, :], hb[:, k * 128:(k + 1) * 128])
                # fp8 copy of h^T for the DoubleRow r/z stationary
                nc.gpsimd.tensor_copy(hT8_sb[lidx][:], hT_sb[lidx][:])

            def y_tail(j, t):
                yp = psum.tile([MB, O], F32, tag="pb", bufs=4, name="ypb")
                for k in range(KT):
                    nc.tensor.matmul(
                        yp[:], hT_sb[1][:, k, :],
                        fcw_sb[:, k * O:(k + 1) * O],
                        start=(k == 0), stop=(k == KT - 1))
                ytmp = wpool.tile([MB, O], F32, tag="ytmp", bufs=2)
                nc.vector.tensor_add(ytmp[:], yp[:], fcb_sb[:])
                nc.scalar.activation(y_cur[:], ytmp[:], AF.Tanh)
                nc.vector.tensor_copy(y_acc[j][:, bass.ts(t, O)], y_cur[:])
                typ = psum.tile([O, MB], F32, tag="pb", bufs=4, name="typb")
                nc.tensor.transpose(typ[:], y_cur[:], ident_sb[:])
                nc.vector.tensor_copy(inp_cur[0:O, :], typ[:])
                nc.sync.dma_start(
                    inp_cur[O:IN, :], xseq_sb[j][:, bass.ts(t, MB)])

            def emit_body(t_of):
                """Emit UNROLL steps with cross-step interleaved PE order."""
                nxt = [None]  # rt/zt preallocated+B-filled for next step L0
                def l0_p1B():
                    rt, zt = p1_alloc()
                    p1_B(0, rt, zt)
                    return rt, zt
                nxt[0] = l0_p1B()
                for j in range(UNROLL):
                    rt0, zt0 = nxt[0]
                    p1_A(0, rt0, zt0)
                    r0, z0 = sigmoids(0, rt0, zt0)
                    An0, Bn0 = p2_B(0)
                    p2_A(0, An0)
                    rt1, zt1 = p1_alloc()
                    p1_B(1, rt1, zt1)             # fills L0 n-chain latency
                    n_chain(0, r0, z0, An0, Bn0)
                    transposes(0)
                    p1_A(1, rt1, zt1)
                    r1, z1 = sigmoids(1, rt1, zt1)
                    An1, Bn1 = p2_B(1)
                    p2_A(1, An1)
                    if j + 1 < UNROLL:
                        nxt[0] = l0_p1B()          # fills L1 n-chain latency
                    n_chain(1, r1, z1, An1, Bn1)
                    transposes(1)
                    y_tail(j, t_of(j))

            if unrolled:
                assert nT <= UNROLL, "unrolled mode supports nT <= UNROLL"
                emit_body(lambda j: 0)
            else:
                assert nT % UNROLL == 0
                with tc.For_i(0, nT // UNROLL, 1,
                              hint_engines=(mybir.EngineType.PE,
                                            mybir.EngineType.DVE,
                                            mybir.EngineType.Activation)) as iv:
                    emit_body(lambda j: iv)

            for j in range(UNROLL):
                nc.sync.dma_start(y_out[j], y_acc[j][:])

    return nc


# ---------------------------------------------------------------------------
# host-side preparation
# ---------------------------------------------------------------------------

def _pair_fp8(wT_rz):
    """[H, RZ] f32 -> [KP, 128, 2, RZ] fp8 DoubleRow pair layout.

    pass p partition j pairs K rows (256p + j, 256p + 128 + j)."""
    out = np.zeros((KP, 128, 2, RZ), np.float32)
    for p in range(KP):
        blk = wT_rz[256 * p:256 * (p + 1)]           # [256, RZ]
        out[p, :, 0, :] = blk[0:128]
        out[p, :, 1, :] = blk[128:256]
    return np.asarray(out * WS, F8E4)


def _prep_in_maps(x, init_noise, w_ih0, w_hh0, b_ih0, b_hh0,
                  w_ih1, w_hh1, b_ih1, b_hh1, fc_w, fc_b, nT):
    f32 = np.float32
    x = np.asarray(x, f32)
    init_noise = np.asarray(init_noise, f32)
    w_ih0 = np.asarray(w_ih0, f32); w_hh0 = np.asarray(w_hh0, f32)
    b_ih0 = np.asarray(b_ih0, f32); b_hh0 = np.asarray(b_hh0, f32)
    w_ih1 = np.asarray(w_ih1, f32); w_hh1 = np.asarray(w_hh1, f32)
    b_ih1 = np.asarray(b_ih1, f32); b_hh1 = np.asarray(b_hh1, f32)
    fc_w = np.asarray(fc_w, f32); fc_b = np.asarray(fc_b, f32)

    # L0 moving operand with bias row: rz bias = b_ih0+b_hh0, n bias = b_ih0.
    # rz columns (and their bias entries) pre-scaled by WS to match the fp8
    # DoubleRow gh0 products accumulated into the same PSUM.
    bias0 = np.concatenate([(b_ih0 + b_hh0)[:RZ], b_ih0[RZ:]])
    wih0_m = np.concatenate([w_ih0.T, bias0[None, :]], 0)        # [11, G]
    wih0_m = wih0_m.copy()
    wih0_m[:, :RZ] *= WS

    whh0rz = _pair_fp8(w_hh0.T[:, :RZ])
    wih1rz = _pair_fp8(w_ih1.T[:, :RZ])
    whh1rz = _pair_fp8(w_hh1.T[:, :RZ])
    whh0n = w_hh0.T[:, RZ:].reshape(KT, 128, H)
    wih1n = w_ih1.T[:, RZ:].reshape(KT, 128, H)
    whh1n = w_hh1.T[:, RZ:].reshape(KT, 128, H)
    fcw_m = fc_w.T.reshape(KT, 128, O)
    biasA1 = np.concatenate(
        [(b_ih1 + b_hh1)[:RZ] * WS, b_ih1[RZ:]])[None, :]
    bhhn0 = b_hh0[RZ:][None, :]
    bhhn1 = b_hh1[RZ:][None, :]
    ones1 = np.ones((1, MB), f32)
    fcb = np.tile(fc_b[None, :], (MB, 1))
    ident = np.eye(MB, dtype=f32)

    noise = init_noise.copy()
    noise[:, -4:] = x[:, 0, -4:]

    in_maps = []
    for c in range(NCORES):
        sl = slice(c * MB, (c + 1) * MB)
        xs = x[sl]            # [MB, T, C]
        # buffer j slot i holds the x-part consumed by step U*i+j+1
        nU = max(nT // UNROLL, 1)
        xseq_h = np.zeros((UNROLL, C, nU * MB), f32)
        for t in range(nT):
            i, j = t // UNROLL, t % UNROLL
            xseq_h[j][:, i * MB:(i + 1) * MB] = xs[:, t, :].T
        inp0v = np.zeros((IN + 1, MB), f32)
        inp0v[IN, :] = 1.0
        inp0v[0:O, :] = 1.0
        inp0v[O:IN, :] = noise[sl].T
        in_maps.append({
            "wih0": wih0_m.astype(BF16),
            "whh0rz": whh0rz, "wih1rz": wih1rz, "whh1rz": whh1rz,
            "whh0n": whh0n.astype(BF16), "wih1n": wih1n.astype(BF16),
            "whh1n": whh1n.astype(BF16),
            "fcw": fcw_m.astype(BF16),
            "biasA1": biasA1.astype(BF16), "bhhn0": bhhn0.astype(BF16),
            "bhhn1": bhhn1.astype(BF16), "ones1": ones1.astype(BF16),
            "fcb": fcb,
            "xseq": xseq_h.astype(BF16), "inp0": inp0v.astype(BF16),
            "ident": ident,
        })
    return in_maps


_BUILD_CACHE = {}

def _get_nc(nT, unrolled):
    key = (nT, unrolled)
    if key not in _BUILD_CACHE:
        nc = build(nT, unrolled)
        nc.finalize()
        _split_excess_waits(nc)
        _BUILD_CACHE[key] = nc
    return _BUILD_CACHE[key]


def run(nT=T, unrolled=False, trace=False, **inputs):
    inputs.pop("xlens", None)
    in_maps = _prep_in_maps(nT=nT, **inputs)
    nc = _get_nc(nT, unrolled)
    res = run_bass_kernel_spmd(nc, in_maps, list(range(NCORES)), trace=trace)
    out = np.zeros((B, nT, O), np.float32)
    nU = max(nT // UNROLL, 1)
    for c in range(NCORES):
        yo = res.results[c]["y_out"].reshape(UNROLL, MB, nU, O)
        for j in range(UNROLL):
            out[c * MB:(c + 1) * MB, j::UNROLL] = yo[j].transpose(0, 1, 2)
    return out, res


def kernel(**inputs):
    out, _ = run(nT=T, unrolled=False, **inputs)
    return out
